# revision 9
# baseline (speedup 1.0000x reference)
"""ConfigurableMamba (Mamba2 x4) forward on 8 Trainium2 NeuronCores.

Strategy: data-parallel over batch (16 samples -> 2 per core), params
replicated (jax.pmap, in_axes=None). The sequential SSM scan is replaced
by the chunked SSD algorithm (chunk Q=128): intra-chunk masked [Q,Q]
matmuls + a 16-step inter-chunk state recurrence -- numerically
equivalent to the reference scan. A pure-NumPy implementation of the
same algorithm is kept as a correctness fallback if the device path
fails for any reason.
"""

import numpy as np

NL = 4
D_MODEL = 256
N_CH = 64
N_CLS = 5
D_INNER = 512
D_STATE = 64
D_CONV = 4
HP = 64
NH = 8
CONV_DIM = 640
BATCH, SEQ = 16, 2048
EPS = 1e-5
QC = 128
NCHUNK = SEQ // QC
NCORES = 8

# ----------------------------------------------------------------------
# jax / Trainium path
# ----------------------------------------------------------------------

_PMAP_FN = None


def _build_pmap_fn():
    import jax
    import jax.numpy as jnp

    tril = np.tril(np.ones((QC, QC), np.float32))

    def _layernorm(h, w, b):
        mu = jnp.mean(h, -1, keepdims=True)
        var = jnp.mean(jnp.square(h - mu), -1, keepdims=True)
        return (h - mu) * jax.lax.rsqrt(var + EPS) * w + b

    def _mamba2(h, W_in, conv_w, conv_b, dt_bias, A_log, Dh, norm_w, W_out):
        Bsz, L = h.shape[0], h.shape[1]
        zxbcdt = h @ W_in                                    # [B,L,1160]
        z = zxbcdt[..., :D_INNER]
        xBC = zxbcdt[..., D_INNER:D_INNER + CONV_DIM]
        dt = zxbcdt[..., D_INNER + CONV_DIM:]

        # causal depthwise conv (kernel 4) + bias + SiLU
        xp = jnp.pad(xBC, ((0, 0), (D_CONV - 1, 0), (0, 0)))
        conv = xp[:, 0:L, :] * conv_w[:, 0]
        for k in range(1, D_CONV):
            conv = conv + xp[:, k:k + L, :] * conv_w[:, k]
        xBC = jax.nn.silu(conv + conv_b)

        xs = xBC[..., :D_INNER]
        Bm = xBC[..., D_INNER:D_INNER + D_STATE]
        Cm = xBC[..., D_INNER + D_STATE:]
        # softplus via plain exp/ln only; written as ln(0.5 + 0.5*e^-|x|)+ln2
        # so XLA cannot re-canonicalize it to log1p/softplus (absent from the
        # neuron ACT function tables).
        dtb = dt + dt_bias
        dt = (jnp.maximum(dtb, 0.0)
              + jnp.log(0.5 + 0.5 * jnp.exp(-jnp.abs(dtb)))
              + 0.6931471805599453)
        A = -jnp.exp(A_log)                                  # [H]

        # ---- chunked SSD (head-major) ----
        xh = xs.reshape(Bsz, NCHUNK, QC, NH, HP).transpose(0, 1, 3, 2, 4)
        dtc = dt.reshape(Bsz, NCHUNK, QC, NH).transpose(0, 1, 3, 2)
        Bc = Bm.reshape(Bsz, NCHUNK, QC, D_STATE)
        Cc = Cm.reshape(Bsz, NCHUNK, QC, D_STATE)

        at = jnp.cumsum(dtc * A[None, None, :, None], axis=-1)   # [B,C,H,Q]
        at_last = at[..., -1]                                    # [B,C,H]

        # intra-chunk: M[i,j] = (C_i.B_j) * exp(at_i - at_j) * dt_j, j<=i
        scores = Cc @ Bc.transpose(0, 1, 3, 2)               # [B,C,Q,Q]
        diff = jnp.clip(at[..., :, None] - at[..., None, :], -80.0, 0.0)
        Lmat = jnp.exp(diff) * tril                          # [B,C,H,Q,Q]
        M = scores[:, :, None] * Lmat * dtc[..., None, :]
        y = M @ xh                                           # [B,C,H,Q,P]

        # chunk state contributions
        w_state = jnp.exp(at_last[..., None] - at) * dtc     # [B,C,H,Q]
        xw = xh * w_state[..., None]
        S_chunk = xw.transpose(0, 1, 2, 4, 3) @ Bc[:, :, None]   # [B,C,H,P,N]
        dA_chunk = jnp.exp(at_last)                          # [B,C,H]
        ea = jnp.exp(at)                                     # [B,C,H,Q]

        # 16-step recurrence; y_inter uses state entering each chunk
        def step(S, inp):
            Cc_c, ea_c, dA_c, Sc_c = inp
            yi = Cc_c[:, None] @ S.transpose(0, 1, 3, 2)     # [B,H,Q,P]
            yi = yi * ea_c[..., None]
            S = dA_c[..., None, None] * S + Sc_c
            return S, yi

        S0 = jnp.zeros((Bsz, NH, HP, D_STATE), jnp.float32)
        inps = (Cc.swapaxes(0, 1), ea.swapaxes(0, 1),
                dA_chunk.swapaxes(0, 1), S_chunk.swapaxes(0, 1))
        _, yis = jax.lax.scan(step, S0, inps)                # [C,B,H,Q,P]
        y = y + yis.swapaxes(0, 1)

        y = y + xh * Dh[None, None, :, None, None]
        y = y.transpose(0, 1, 3, 2, 4).reshape(Bsz, L, D_INNER)

        # gated RMSNorm + out-proj
        y = y * jax.nn.silu(z)
        y = y * jax.lax.rsqrt(
            jnp.mean(jnp.square(y), -1, keepdims=True) + EPS) * norm_w
        return y @ W_out

    def model(x, lin_in_w, lin_in_b, W_in, conv_w, conv_b, dt_bias, A_log,
              Dp, norm_w, W_out, ln_w, ln_b, lin_out_w, lin_out_b):
        h = x @ lin_in_w + lin_in_b

        def layer(h, p):
            (W_in_i, conv_w_i, conv_b_i, dt_bias_i, A_log_i, Dp_i,
             norm_w_i, W_out_i, ln_w_i, ln_b_i) = p
            m = _mamba2(h, W_in_i, conv_w_i, conv_b_i, dt_bias_i,
                        A_log_i, Dp_i, norm_w_i, W_out_i)
            return _layernorm(m + h, ln_w_i, ln_b_i), None

        h, _ = jax.lax.scan(
            layer, h,
            (W_in, conv_w, conv_b, dt_bias, A_log, Dp, norm_w, W_out,
             ln_w, ln_b))
        return h @ lin_out_w + lin_out_b

    return jax.pmap(model, in_axes=0)


_DEV_PARAMS = None


def _device_kernel(x, params):
    global _PMAP_FN, _DEV_PARAMS
    import jax
    if _PMAP_FN is None:
        _PMAP_FN = _build_pmap_fn()
    if _DEV_PARAMS is None:
        devs = jax.devices()[:NCORES]
        _DEV_PARAMS = [jax.device_put_replicated(p, devs) for p in params]
    xs = x.reshape(NCORES, BATCH // NCORES, SEQ, N_CH)
    out = _PMAP_FN(xs, *_DEV_PARAMS)
    return np.asarray(out).reshape(BATCH, SEQ, N_CLS).astype(np.float32)


# ----------------------------------------------------------------------
# NumPy fallback (same chunked-SSD algorithm)
# ----------------------------------------------------------------------

def _silu(x):
    return x / (1.0 + np.exp(-x))


def _softplus(x):
    return np.logaddexp(0.0, x)


def _np_layernorm(h, w, b):
    mu = h.mean(-1, keepdims=True)
    var = np.square(h - mu).mean(-1, keepdims=True)
    return (h - mu) / np.sqrt(var + EPS) * w + b


def _np_mamba2(h, W_in, conv_w, conv_b, dt_bias, A_log, Dh, norm_w, W_out):
    Bsz, L, _ = h.shape
    zxbcdt = h.reshape(-1, D_MODEL) @ W_in
    zxbcdt = zxbcdt.reshape(Bsz, L, -1)
    z = zxbcdt[:, :, :D_INNER]
    xBC = zxbcdt[:, :, D_INNER:D_INNER + CONV_DIM]
    dt = zxbcdt[:, :, D_INNER + CONV_DIM:]

    xp = np.pad(xBC, ((0, 0), (D_CONV - 1, 0), (0, 0)))
    conv = xp[:, 0:L, :] * conv_w[:, 0][None, None, :]
    for k in range(1, D_CONV):
        conv += xp[:, k:k + L, :] * conv_w[:, k][None, None, :]
    xBC = _silu(conv + conv_b)

    xs = xBC[:, :, :D_INNER]
    Bm = np.ascontiguousarray(xBC[:, :, D_INNER:D_INNER + D_STATE])
    Cm = np.ascontiguousarray(xBC[:, :, D_INNER + D_STATE:])
    dt = _softplus(dt + dt_bias)
    A = -np.exp(A_log)

    xh = np.ascontiguousarray(
        xs.reshape(Bsz, NCHUNK, QC, NH, HP).transpose(0, 1, 3, 2, 4))
    dtc = dt.reshape(Bsz, NCHUNK, QC, NH).transpose(0, 1, 3, 2)
    Bc = Bm.reshape(Bsz, NCHUNK, QC, D_STATE)
    Cc = Cm.reshape(Bsz, NCHUNK, QC, D_STATE)

    at = np.cumsum(dtc * A[None, None, :, None], axis=-1)
    at_last = at[..., -1]

    scores = np.matmul(Cc, Bc.transpose(0, 1, 3, 2))
    diff = at[..., :, None] - at[..., None, :]
    np.clip(diff, -80.0, 0.0, out=diff)
    Lmat = np.exp(diff)
    Lmat *= np.tril(np.ones((QC, QC), np.float32))
    M = scores[:, :, None] * Lmat * dtc[..., None, :]
    y = np.matmul(M, xh)

    w_state = np.exp(at_last[..., None] - at) * dtc
    xw = xh * w_state[..., None]
    S_chunk = np.matmul(xw.transpose(0, 1, 2, 4, 3), Bc[:, :, None])
    dA_chunk = np.exp(at_last)

    ea = np.exp(at)
    S = np.zeros((Bsz, NH, HP, D_STATE), np.float32)
    for c in range(NCHUNK):
        yi = np.matmul(Cc[:, c, None], S.transpose(0, 1, 3, 2))
        y[:, c] += yi * ea[:, c, :, :, None]
        S = dA_chunk[:, c, :, None, None] * S + S_chunk[:, c]

    y += xh * Dh[None, None, :, None, None]
    y = y.transpose(0, 1, 3, 2, 4).reshape(Bsz, L, D_INNER)

    y = y * _silu(z)
    y = y / np.sqrt(np.square(y).mean(-1, keepdims=True) + EPS) * norm_w
    return (y.reshape(-1, D_INNER) @ W_out).reshape(Bsz, L, D_MODEL)


def _numpy_kernel(x, lin_in_w, lin_in_b, W_in, conv_w, conv_b, dt_bias,
                  A_log, Dp, norm_w, W_out, ln_w, ln_b, lin_out_w, lin_out_b):
    h = x.reshape(-1, N_CH) @ lin_in_w + lin_in_b
    h = h.reshape(BATCH, SEQ, D_MODEL)
    for i in range(NL):
        m = _np_mamba2(h, W_in[i], conv_w[i], conv_b[i], dt_bias[i],
                       A_log[i], Dp[i], norm_w[i], W_out[i])
        h = _np_layernorm(m + h, ln_w[i], ln_b[i])
    out = h.reshape(-1, D_MODEL) @ lin_out_w + lin_out_b
    return out.reshape(BATCH, SEQ, N_CLS).astype(np.float32)


# ----------------------------------------------------------------------
# entry point
# ----------------------------------------------------------------------

def kernel(x, lin_in_w, lin_in_b, W_in, conv_w, conv_b, dt_bias, A_log, Dp,
           norm_w, W_out, ln_w, ln_b, lin_out_w, lin_out_b):
    x = np.asarray(x, np.float32)
    params = [np.asarray(a, np.float32) for a in
              (lin_in_w, lin_in_b, W_in, conv_w, conv_b, dt_bias, A_log, Dp,
               norm_w, W_out, ln_w, ln_b, lin_out_w, lin_out_b)]
    try:
        return _device_kernel(x, params)
    except Exception:
        import sys
        import traceback
        traceback.print_exc(file=sys.stderr)
        return _numpy_kernel(x, *params)
